# revision 8
# baseline (speedup 1.0000x reference)
"""Trainium2 Bass kernel for the CS224N 2-layer RNN language model.

Model: x = emb[idx]; two tanh-RNN layers; logits = y1 @ W_out.T + b_out.
Returns (output_prob [B*T,V], rnn_hidden [2,B,H], last_output [B,V]).

Strategy (8 NeuronCores, SPMD):
  - The RNN (embedding gather, both input projections, both recurrences)
    is replicated on every core: the sequential recurrence is bound by
    streaming W_hh through the PE array each step, which cannot be
    usefully split across cores without per-step collectives (measured
    ~21us latency per collective here - prohibitive).
  - The dominant [4096,1024]@[1024,32000] logits GEMM is tensor-parallel
    sharded over the vocab dim: W_out is padded to 32768 cols and each
    core computes a 4096-col shard. Host concatenates shards.
  - All GEMMs run as float32r (TF32-like, 1 cycle/row at N>=512;
    measured mean rel err ~1e-3, vs 4 cycles/row for full fp32).
  - Internal time-major layout (row = t*B + b) so per-step slices and
    projection tiles are contiguous; transposed activations (hT blocks)
    are produced on the PE (transpose mode) each step and double as both
    the next step's stationary operand and the next layer's input.

Self-contained: hardcodes all shapes; no sibling imports.
"""

import numpy as np

V, E, H, B, T = 32000, 1024, 1024, 32, 128
VPAD = 32768
NCORES = 8
VSH = VPAD // NCORES  # 4096 per-core vocab shard

# ---------------------------------------------------------------------------
# BIR post-pass: the walrus build in this image encodes at most ONE semaphore
# wait per instruction; Tile attaches many to its kernel-tail drain. Move the
# excess onto NoOps inserted immediately before, same engine (same semantics).
# ---------------------------------------------------------------------------
_wsplit_counter = [0]


def _split_excess_waits(nc, limit=1):
    import bass_rust
    import concourse.mybir as mybir

    for func in nc.m.functions:
        for bb in func.blocks:
            il = bb.instructions
            i = 0
            while i < len(il):
                inst = il[i]
                si = getattr(inst, "sync_info", None)
                if si is None:
                    i += 1
                    continue
                w = list(si.on_wait)
                if len(w) > limit:
                    keep = w[-limit:]
                    excess = w[:-limit]
                    si.on_wait = keep
                    ncarr = 0
                    for j in range(0, len(excess), limit):
                        _wsplit_counter[0] += 1
                        nop = mybir.InstNoOp(
                            name=f"WSPLIT-{_wsplit_counter[0]}", ins=[], outs=[]
                        )
                        nop.engine = inst.engine
                        nop.sync_info = bass_rust.SyncInfo(
                            on_wait=excess[j : j + limit], on_update=[]
                        )
                        il.insert(i + ncarr, nop)
                        ncarr += 1
                    i += ncarr
                i += 1


# ---------------------------------------------------------------------------
# Device program
# ---------------------------------------------------------------------------


def _build_nc(nsteps=T, debug_outputs=False):
    import concourse.bass as bass
    import concourse.mybir as mybir
    import concourse.tile as tile
    from concourse.masks import make_identity

    F32 = mybir.dt.float32
    F32R = mybir.dt.float32r
    I32 = mybir.dt.int32
    Tanh = mybir.ActivationFunctionType.Tanh

    NT = nsteps
    BTn = NT * B  # rows, time-major
    NPROJ = BTn // 128  # projection tiles (128 rows each)
    RING0 = 512  # cols in y0T ring (16 steps of history)
    RING1 = 512
    NBTG = (BTn + 511) // 512  # logits bt groups of <=512 rows
    NVC = 4  # logits vocab chunks of 1024

    nc = bass.Bass()

    okind = "ExternalOutput" if debug_outputs else "Internal"

    idx_d = nc.dram_tensor("idx_t", [BTn, 1], I32, kind="ExternalInput")
    emb_d = nc.dram_tensor("emb", [V, E], F32, kind="ExternalInput")
    hT_init_d = nc.dram_tensor("hT_init", [128, 2, 8, 32], F32R, kind="ExternalInput")
    wih_d = [
        nc.dram_tensor(f"wihT{l}", [E, H], F32R, kind="ExternalInput") for l in (0, 1)
    ]
    whh_d = [
        nc.dram_tensor(f"whhT{l}", [H, H], F32R, kind="ExternalInput") for l in (0, 1)
    ]
    bias_d = [
        nc.dram_tensor(f"bias{l}", [128, H], F32, kind="ExternalInput") for l in (0, 1)
    ]
    wout_d = nc.dram_tensor("woutT", [H, VSH], F32R, kind="ExternalInput")
    bout_d = nc.dram_tensor("bout", [128, VSH], F32, kind="ExternalInput")

    logits_d = nc.dram_tensor("logits", [BTn, VSH], F32, kind="ExternalOutput")
    rnnh_d = nc.dram_tensor("rnnh", [2, 128, 8, 32], F32, kind="ExternalOutput")

    xw_d = [nc.dram_tensor(f"xw{l}", [BTn, H], F32, kind=okind) for l in (0, 1)]
    y1t_d = nc.dram_tensor("y1t", [8, 128, BTn], F32R, kind=okind)

    # output rows are b-major (row b*NT + t) to match the reference layout
    logits_bt = logits_d[:].rearrange("(b t) v -> b t v", t=NT)

    with tile.TileContext(nc) as tc:
        with (
            tc.tile_pool(name="const", bufs=1) as constp,
            tc.tile_pool(name="wpool", bufs=1) as wpool,
            tc.tile_pool(name="stream", bufs=1) as stream,
            tc.tile_pool(name="ps", bufs=1, space="PSUM") as ps,
        ):
            ident = constp.tile([128, 128], F32)
            make_identity(nc, ident[:])
            ring = [
                constp.tile([128, 8, RING0], F32R, name="ring0"),
                constp.tile([128, 8, RING1], F32R, name="ring1"),
            ]
            hT0 = constp.tile([128, 2, 8, 32], F32R, name="hT0")
            nc.sync.dma_start(out=hT0[:], in_=hT_init_d[:])
            biasl = []
            for l in (0, 1):
                bl = constp.tile([128, H], F32, name=f"biasl{l}")
                nc.sync.dma_start(out=bl[:], in_=bias_d[l][:])
                biasl.append(bl)

            def wbig_tile(name):
                return wpool.tile([128, 8, 1024], F32R, name=name, tag="wbig", bufs=2)

            def whh_tile(name):
                return wpool.tile([128, 8, 1024], F32R, name=name, tag="whh", bufs=1)

            def psum_c(jc, mrows=128):
                return ps.tile(
                    [mrows, 512], mybir.dt.float32, space="PSUM",
                    tag=f"pc{jc}", name=f"pc{jc}", bufs=2,
                )

            def psum_t(g):
                return ps.tile(
                    [128, 512], mybir.dt.float32, space="PSUM",
                    tag=f"pt{g}", name=f"pt{g}", bufs=1,
                )

            def psum_q(jc):
                return ps.tile(
                    [128, 512], mybir.dt.float32, space="PSUM",
                    tag=f"pq{jc}", name=f"pq{jc}", bufs=1,
                )

            def proj_tile(k, stat, w_sb, bias_sb, out_dram):
                """out rows k*128..k*128+128 = stat.T @ w  (+bias)."""
                pcs = [psum_c(0), psum_c(1)]
                for i in range(8):
                    for jc in (0, 1):
                        nc.tensor.matmul(
                            pcs[jc][:],
                            stat(i),
                            w_sb[:, i, jc * 512 : (jc + 1) * 512],
                            start=(i == 0),
                            stop=(i == 7),
                        )
                out_sb = stream.tile(
                    [128, 1024], F32, name="projout", tag="buf4k", bufs=3
                )
                for jc in (0, 1):
                    nc.vector.tensor_add(
                        out=out_sb[:, jc * 512 : (jc + 1) * 512],
                        in0=pcs[jc][:],
                        in1=bias_sb[:, jc * 512 : (jc + 1) * 512],
                    )
                nc.sync.dma_start(
                    out=out_dram[k * 128 : (k + 1) * 128, :], in_=out_sb[:]
                )

            # =============== P1: embedding gather + L0 input projection
            _scope = nc.named_scope("P1_proj0")
            _scope.__enter__()
            wih0_sb = wbig_tile("wih0_sb")
            nc.sync.dma_start(
                out=wih0_sb[:], in_=wih_d[0][:].rearrange("(kb p) n -> p kb n", p=128)
            )
            for k in range(NPROJ):
                idx_sb = stream.tile([128, 1], I32, name="idx_sb", tag="idx", bufs=2)
                nc.sync.dma_start(out=idx_sb[:], in_=idx_d[k * 128 : (k + 1) * 128, :])
                xg = stream.tile([128, 1024], F32, name="xg", tag="buf4k", bufs=3)
                nc.gpsimd.indirect_dma_start(
                    out=xg[:],
                    out_offset=None,
                    in_=emb_d[:],
                    in_offset=bass.IndirectOffsetOnAxis(ap=idx_sb[:, :1], axis=0),
                )
                xT = stream.tile([128, 8, 512], F32R, name="xT", tag="t8x", bufs=2)
                for g in (0, 1):
                    pt = psum_t(g)
                    for i in range(4):
                        blk = g * 4 + i
                        nc.tensor.transpose(
                            out=pt[:, i * 128 : (i + 1) * 128],
                            in_=xg[:, blk * 128 : (blk + 1) * 128],
                            identity=ident[:],
                        )
                    nc.vector.tensor_copy(
                        out=xT[:, g * 4 : (g + 1) * 4, :128],
                        in_=pt[:].rearrange("p (a b) -> p a b", a=4),
                    )
                proj_tile(
                    k,
                    lambda i, xT=xT: xT[:, i, :128],
                    wih0_sb,
                    biasl[0][:],
                    xw_d[0],
                )

            _scope.__exit__(None, None, None)

            # =============== P2/P3: recurrences (+ interleaved L1 projection)
            wih1_sb = wbig_tile("wih1_sb")
            nc.sync.dma_start(
                out=wih1_sb[:], in_=wih_d[1][:].rearrange("(kb p) n -> p kb n", p=128)
            )

            for layer in (0, 1):
                _scope = nc.named_scope(f"P{2 + layer}_rec{layer}")
                _scope.__enter__()
                rng = ring[layer]
                rcols = RING0 if layer == 0 else RING1
                whh_sb = whh_tile(f"whh{layer}_sb")
                nc.sync.dma_start(
                    out=whh_sb[:],
                    in_=whh_d[layer][:].rearrange("(kb p) n -> p kb n", p=128),
                )
                for t in range(NT):
                    xw_t = stream.tile([32, 1024], F32, name="xw_t", tag="xwin", bufs=2)
                    nc.sync.dma_start(
                        out=xw_t[:], in_=xw_d[layer][t * 32 : (t + 1) * 32, :]
                    )
                    pcs = [psum_c(0, 32), psum_c(1, 32)]
                    rc_prev = ((t - 1) * 32) % rcols
                    for i in range(8):
                        stat = (
                            hT0[:, layer, i, :]
                            if t == 0
                            else rng[:, i, rc_prev : rc_prev + 32]
                        )
                        for jc in (0, 1):
                            nc.tensor.matmul(
                                pcs[jc][:],
                                stat,
                                whh_sb[:, i, jc * 512 : (jc + 1) * 512],
                                start=(i == 0),
                                stop=(i == 7),
                            )
                    h_sb = stream.tile([32, 1024], F32, name="h_sb", tag="h", bufs=2)
                    for jc in (0, 1):
                        nc.vector.tensor_add(
                            out=h_sb[:, jc * 512 : (jc + 1) * 512],
                            in0=pcs[jc][:],
                            in1=xw_t[:, jc * 512 : (jc + 1) * 512],
                        )
                        nc.scalar.activation(
                            h_sb[:, jc * 512 : (jc + 1) * 512],
                            h_sb[:, jc * 512 : (jc + 1) * 512],
                            Tanh,
                        )
                    rc = (t * 32) % rcols
                    for g in (0, 1):
                        pt = psum_t(g)
                        for i in range(4):
                            blk = g * 4 + i
                            nc.tensor.transpose(
                                out=pt[:, i * 32 : (i + 1) * 32],
                                in_=h_sb[:, blk * 128 : (blk + 1) * 128],
                                identity=ident[:32, :32],
                            )
                        nc.vector.tensor_copy(
                            out=rng[:, g * 4 : (g + 1) * 4, rc : rc + 32],
                            in_=pt[:, : 4 * 32].rearrange("p (a b) -> p a b", a=4),
                        )
                    if layer == 0 and t % 4 == 3:
                        k = t // 4
                        base = (k * 128) % RING0
                        proj_tile(
                            k,
                            lambda i, base=base: rng[:, i, base : base + 128],
                            wih1_sb,
                            biasl[1][:],
                            xw_d[1],
                        )
                    if layer == 1 and t % 8 == 7:
                        dbase = ((t - 7) * 32) % RING1
                        gbase = (t - 7) * 32
                        for blk in range(8):
                            nc.sync.dma_start(
                                out=y1t_d[blk, :, gbase : gbase + 256],
                                in_=rng[:, blk, dbase : dbase + 256],
                            )
                # final hidden capture (cast F32R ring -> F32 out)
                fc = ((NT - 1) * 32) % rcols
                hcap = stream.tile([128, 8, 32], F32, name="hcap", tag="hcap", bufs=1)
                nc.vector.tensor_copy(
                    out=hcap[:], in_=rng[:, :, fc : fc + 32]
                )
                nc.sync.dma_start(out=rnnh_d[layer], in_=hcap[:])
                _scope.__exit__(None, None, None)

            # =============== P4: logits GEMM (vocab shard)
            _scope = nc.named_scope("P4_logits")
            _scope.__enter__()
            for vc in range(NVC):
                wout_sb = wbig_tile("wout_sb")
                nc.sync.dma_start(
                    out=wout_sb[:],
                    in_=wout_d[:, vc * 1024 : (vc + 1) * 1024].rearrange(
                        "(kb p) n -> p kb n", p=128
                    ),
                )
                bout_sb = stream.tile(
                    [128, 1024], F32, name="bout_sb", tag="bout", bufs=1
                )
                nc.sync.dma_start(
                    out=bout_sb[:], in_=bout_d[:, vc * 1024 : (vc + 1) * 1024]
                )
                for btg in range(NBTG):
                    gw = min(512, BTn - btg * 512)
                    y1tile = stream.tile(
                        [128, 8, 512], F32R, name="y1tile", tag="t8x", bufs=2
                    )
                    nc.sync.dma_start(
                        out=y1tile[:, :, :gw],
                        in_=y1t_d[:, :, btg * 512 : btg * 512 + gw].rearrange(
                            "blk p c -> p blk c"
                        ),
                    )
                    for sub in range(gw // 128):
                        pcs = [psum_q(0), psum_q(1)]
                        for i in range(8):
                            stat = y1tile[:, i, sub * 128 : (sub + 1) * 128]
                            for jc in (0, 1):
                                nc.tensor.matmul(
                                    pcs[jc][:],
                                    stat,
                                    wout_sb[:, i, jc * 512 : (jc + 1) * 512],
                                    start=(i == 0),
                                    stop=(i == 7),
                                )
                        out_sb = stream.tile(
                            [128, 1024], F32, name="lout", tag="buf4k", bufs=3
                        )
                        for jc in (0, 1):
                            nc.vector.tensor_add(
                                out=out_sb[:, jc * 512 : (jc + 1) * 512],
                                in0=pcs[jc][:],
                                in1=bout_sb[:, jc * 512 : (jc + 1) * 512],
                            )
                        # rows are time-major t*32+b; store to b-major rows
                        lt = btg * 4 + sub  # 128-row tile index
                        for tt in range(4):
                            trow = lt * 4 + tt
                            nc.sync.dma_start(
                                out=logits_bt[:, trow, vc * 1024 : (vc + 1) * 1024],
                                in_=out_sb[tt * 32 : (tt + 1) * 32, :],
                            )
            _scope.__exit__(None, None, None)
    return nc


# ---------------------------------------------------------------------------
# Host entry point
# ---------------------------------------------------------------------------


def _prep_inputs(input_x, hidden, emb, W_ih0, W_hh0, b_ih0, b_hh0,
                 W_ih1, W_hh1, b_ih1, b_hh1, W_out, b_out, nsteps=T):
    f32 = np.float32
    NT = nsteps
    idx_t = np.ascontiguousarray(
        np.asarray(input_x).astype(np.int32)[:, :NT].T.reshape(-1, 1)
    )
    # hT_init[p, l, blk, b] must equal hidden[l, b, blk*128 + p]
    hT_init = np.ascontiguousarray(
        np.asarray(hidden, f32).transpose(0, 2, 1).reshape(2, 8, 128, 32)
        .transpose(2, 0, 1, 3)
    )
    common = {
        "idx_t": idx_t,
        "emb": np.ascontiguousarray(np.asarray(emb, f32)),
        "hT_init": np.ascontiguousarray(hT_init),
        "wihT0": np.ascontiguousarray(np.asarray(W_ih0, f32).T),
        "whhT0": np.ascontiguousarray(np.asarray(W_hh0, f32).T),
        "wihT1": np.ascontiguousarray(np.asarray(W_ih1, f32).T),
        "whhT1": np.ascontiguousarray(np.asarray(W_hh1, f32).T),
        "bias0": np.ascontiguousarray(
            np.tile((np.asarray(b_ih0, f32) + np.asarray(b_hh0, f32))[None, :],
                    (128, 1))
        ),
        "bias1": np.ascontiguousarray(
            np.tile((np.asarray(b_ih1, f32) + np.asarray(b_hh1, f32))[None, :],
                    (128, 1))
        ),
    }
    woutT_pad = np.zeros((H, VPAD), f32)
    woutT_pad[:, :V] = np.asarray(W_out, f32).T
    bout_pad = np.zeros(VPAD, f32)
    bout_pad[:V] = np.asarray(b_out, f32)
    in_maps = []
    for c in range(NCORES):
        m = dict(common)
        m["woutT"] = np.ascontiguousarray(woutT_pad[:, c * VSH : (c + 1) * VSH])
        m["bout"] = np.ascontiguousarray(
            np.tile(bout_pad[None, c * VSH : (c + 1) * VSH], (128, 1))
        )
        in_maps.append(m)
    return in_maps


_NC_CACHE = {}


def _enable_ldw_opt():
    """Compile with walrus LDWEIGHTS double-buffer optimization enabled.

    bass_utils hardcodes --enable-ldw-opt=false; ~40% of PE time in this
    kernel is weight loads, which the optimization overlaps with matmuls.
    Verified numerically identical on this workload.
    """
    import concourse.bass_utils as bu

    if getattr(bu.run_command, "_ldw_patched", False):
        return
    orig = bu.run_command

    def patched(argv, **kwargs):
        argv = [
            a.replace("--enable-ldw-opt=false", "--enable-ldw-opt=true")
            if isinstance(a, str) else a
            for a in argv
        ]
        return orig(argv, **kwargs)

    patched._ldw_patched = True
    bu.run_command = patched


def kernel(input_x, hidden, emb, W_ih0, W_hh0, b_ih0, b_hh0,
           W_ih1, W_hh1, b_ih1, b_hh1, W_out, b_out):
    from concourse.bass_utils import run_bass_kernel_spmd

    _enable_ldw_opt()

    in_maps = _prep_inputs(
        input_x, hidden, emb, W_ih0, W_hh0, b_ih0, b_hh0,
        W_ih1, W_hh1, b_ih1, b_hh1, W_out, b_out,
    )
    if "nc" not in _NC_CACHE:
        nc = _build_nc()
        _split_excess_waits(nc)
        _NC_CACHE["nc"] = nc
    nc = _NC_CACHE["nc"]
    res = run_bass_kernel_spmd(nc, in_maps, list(range(NCORES)), trace=False)
    results = res.results

    output_prob = np.concatenate(
        [results[c]["logits"] for c in range(NCORES)], axis=1
    )[:, :V]
    rnnh = results[0]["rnnh"]  # [2, 128, 8, 32] = [l, p, blk, b]
    rnn_hidden = np.ascontiguousarray(
        rnnh.transpose(0, 3, 2, 1).reshape(2, B, H)
    )
    last_output = np.ascontiguousarray(
        output_prob.reshape(B, T, V)[:, -1, :]
    )
    return (output_prob, rnn_hidden, last_output)


# revision 9
# speedup vs baseline: 1.0102x; 1.0102x over previous
"""Trainium2 Bass kernel for the CS224N 2-layer RNN language model.

Model: x = emb[idx]; two tanh-RNN layers; logits = y1 @ W_out.T + b_out.
Returns (output_prob [B*T,V], rnn_hidden [2,B,H], last_output [B,V]).

Strategy (8 NeuronCores, SPMD):
  - The RNN (embedding gather, both input projections, both recurrences)
    is replicated on every core: the sequential recurrence is bound by
    streaming W_hh through the PE array each step, which cannot be
    usefully split across cores without per-step collectives (measured
    ~21us latency per collective here - prohibitive).
  - The dominant [4096,1024]@[1024,32000] logits GEMM is tensor-parallel
    sharded over the vocab dim: W_out is padded to 32768 cols and each
    core computes a 4096-col shard. Host concatenates shards.
  - All GEMMs run as float32r (TF32-like, 1 cycle/row at N>=512;
    measured mean rel err ~1e-3, vs 4 cycles/row for full fp32).
  - Internal time-major layout (row = t*B + b) so per-step slices and
    projection tiles are contiguous; transposed activations (hT blocks)
    are produced on the PE (transpose mode) each step and double as both
    the next step's stationary operand and the next layer's input.

Self-contained: hardcodes all shapes; no sibling imports.
"""

import numpy as np

V, E, H, B, T = 32000, 1024, 1024, 32, 128
VPAD = 32768
NCORES = 8
VSH = VPAD // NCORES  # 4096 per-core vocab shard

# ---------------------------------------------------------------------------
# BIR post-pass: the walrus build in this image encodes at most ONE semaphore
# wait per instruction; Tile attaches many to its kernel-tail drain. Move the
# excess onto NoOps inserted immediately before, same engine (same semantics).
# ---------------------------------------------------------------------------
_wsplit_counter = [0]


def _split_excess_waits(nc, limit=1):
    import bass_rust
    import concourse.mybir as mybir

    for func in nc.m.functions:
        for bb in func.blocks:
            il = bb.instructions
            i = 0
            while i < len(il):
                inst = il[i]
                si = getattr(inst, "sync_info", None)
                if si is None:
                    i += 1
                    continue
                w = list(si.on_wait)
                if len(w) > limit:
                    keep = w[-limit:]
                    excess = w[:-limit]
                    si.on_wait = keep
                    ncarr = 0
                    for j in range(0, len(excess), limit):
                        _wsplit_counter[0] += 1
                        nop = mybir.InstNoOp(
                            name=f"WSPLIT-{_wsplit_counter[0]}", ins=[], outs=[]
                        )
                        nop.engine = inst.engine
                        nop.sync_info = bass_rust.SyncInfo(
                            on_wait=excess[j : j + limit], on_update=[]
                        )
                        il.insert(i + ncarr, nop)
                        ncarr += 1
                    i += ncarr
                i += 1


# ---------------------------------------------------------------------------
# Device program
# ---------------------------------------------------------------------------


def _build_nc(nsteps=T, debug_outputs=False):
    import concourse.bass as bass
    import concourse.mybir as mybir
    import concourse.tile as tile
    from concourse.masks import make_identity

    F32 = mybir.dt.float32
    F32R = mybir.dt.float32r
    I32 = mybir.dt.int32
    Tanh = mybir.ActivationFunctionType.Tanh

    NT = nsteps
    BTn = NT * B  # rows, time-major
    NPROJ = BTn // 128  # projection tiles (128 rows each)
    RING0 = 512  # cols in y0T ring (16 steps of history)
    RING1 = 512
    NBTG = (BTn + 511) // 512  # logits bt groups of <=512 rows
    NVC = 4  # logits vocab chunks of 1024

    nc = bass.Bass()

    okind = "ExternalOutput" if debug_outputs else "Internal"

    idx_d = nc.dram_tensor("idx_t", [BTn, 1], I32, kind="ExternalInput")
    emb_d = nc.dram_tensor("emb", [V, E], F32, kind="ExternalInput")
    hT_init_d = nc.dram_tensor("hT_init", [128, 2, 8, 32], F32R, kind="ExternalInput")
    wih_d = [
        nc.dram_tensor(f"wihT{l}", [E, H], F32R, kind="ExternalInput") for l in (0, 1)
    ]
    whh_d = [
        nc.dram_tensor(f"whhT{l}", [H, H], F32R, kind="ExternalInput") for l in (0, 1)
    ]
    bias_d = [
        nc.dram_tensor(f"bias{l}", [128, H], F32, kind="ExternalInput") for l in (0, 1)
    ]
    wout_d = nc.dram_tensor("woutT", [H, VSH], F32R, kind="ExternalInput")
    bout_d = nc.dram_tensor("bout", [128, VSH], F32, kind="ExternalInput")

    logits_d = nc.dram_tensor("logits", [BTn, VSH], F32, kind="ExternalOutput")
    rnnh_d = nc.dram_tensor("rnnh", [2, 128, 8, 32], F32, kind="ExternalOutput")

    xw_d = [nc.dram_tensor(f"xw{l}", [BTn, H], F32, kind=okind) for l in (0, 1)]
    y1t_d = nc.dram_tensor("y1t", [8, 128, BTn], F32R, kind=okind)

    # output rows are b-major (row b*NT + t) to match the reference layout
    logits_bt = logits_d[:].rearrange("(b t) v -> b t v", t=NT)

    with tile.TileContext(nc) as tc:
        with (
            tc.tile_pool(name="const", bufs=1) as constp,
            tc.tile_pool(name="wpool", bufs=1) as wpool,
            tc.tile_pool(name="stream", bufs=1) as stream,
            tc.tile_pool(name="ps", bufs=1, space="PSUM") as ps,
        ):
            ident = constp.tile([128, 128], F32)
            make_identity(nc, ident[:])
            ring = [
                constp.tile([128, 8, RING0], F32R, name="ring0"),
                constp.tile([128, 8, RING1], F32R, name="ring1"),
            ]
            hT0 = constp.tile([128, 2, 8, 32], F32R, name="hT0")
            nc.sync.dma_start(out=hT0[:], in_=hT_init_d[:])
            biasl = []
            for l in (0, 1):
                bl = constp.tile([128, H], F32, name=f"biasl{l}")
                nc.sync.dma_start(out=bl[:], in_=bias_d[l][:])
                biasl.append(bl)

            def wbig_tile(name):
                return wpool.tile([128, 8, 1024], F32R, name=name, tag="wbig", bufs=2)

            def whh_tile(name):
                return wpool.tile([128, 8, 1024], F32R, name=name, tag="whh", bufs=1)

            def psum_c(jc, mrows=128):
                return ps.tile(
                    [mrows, 512], mybir.dt.float32, space="PSUM",
                    tag=f"pc{jc}", name=f"pc{jc}", bufs=2,
                )

            def psum_t(g):
                return ps.tile(
                    [128, 512], mybir.dt.float32, space="PSUM",
                    tag=f"pt{g}", name=f"pt{g}", bufs=2,
                )

            def proj_tile(k, stat, w_sb, bias_sb, out_dram):
                """out rows k*128..k*128+128 = stat.T @ w  (+bias)."""
                pcs = [psum_c(0), psum_c(1)]
                for i in range(8):
                    for jc in (0, 1):
                        nc.tensor.matmul(
                            pcs[jc][:],
                            stat(i),
                            w_sb[:, i, jc * 512 : (jc + 1) * 512],
                            start=(i == 0),
                            stop=(i == 7),
                        )
                out_sb = stream.tile(
                    [128, 1024], F32, name="projout", tag="buf4k", bufs=3
                )
                for jc in (0, 1):
                    nc.vector.tensor_add(
                        out=out_sb[:, jc * 512 : (jc + 1) * 512],
                        in0=pcs[jc][:],
                        in1=bias_sb[:, jc * 512 : (jc + 1) * 512],
                    )
                nc.sync.dma_start(
                    out=out_dram[k * 128 : (k + 1) * 128, :], in_=out_sb[:]
                )

            # =============== P1: embedding gather + L0 input projection
            _scope = nc.named_scope("P1_proj0")
            _scope.__enter__()
            wih0_sb = wbig_tile("wih0_sb")
            nc.sync.dma_start(
                out=wih0_sb[:], in_=wih_d[0][:].rearrange("(kb p) n -> p kb n", p=128)
            )
            for k in range(NPROJ):
                idx_sb = stream.tile([128, 1], I32, name="idx_sb", tag="idx", bufs=2)
                nc.sync.dma_start(out=idx_sb[:], in_=idx_d[k * 128 : (k + 1) * 128, :])
                xg = stream.tile([128, 1024], F32, name="xg", tag="buf4k", bufs=3)
                nc.gpsimd.indirect_dma_start(
                    out=xg[:],
                    out_offset=None,
                    in_=emb_d[:],
                    in_offset=bass.IndirectOffsetOnAxis(ap=idx_sb[:, :1], axis=0),
                )
                xT = stream.tile([128, 8, 512], F32R, name="xT", tag="t8x", bufs=2)
                for g in (0, 1):
                    pt = psum_t(g)
                    for i in range(4):
                        blk = g * 4 + i
                        nc.tensor.transpose(
                            out=pt[:, i * 128 : (i + 1) * 128],
                            in_=xg[:, blk * 128 : (blk + 1) * 128],
                            identity=ident[:],
                        )
                    nc.vector.tensor_copy(
                        out=xT[:, g * 4 : (g + 1) * 4, :128],
                        in_=pt[:].rearrange("p (a b) -> p a b", a=4),
                    )
                proj_tile(
                    k,
                    lambda i, xT=xT: xT[:, i, :128],
                    wih0_sb,
                    biasl[0][:],
                    xw_d[0],
                )

            _scope.__exit__(None, None, None)

            # =============== P2/P3: recurrences (+ interleaved L1 projection)
            wih1_sb = wbig_tile("wih1_sb")
            nc.sync.dma_start(
                out=wih1_sb[:], in_=wih_d[1][:].rearrange("(kb p) n -> p kb n", p=128)
            )

            for layer in (0, 1):
                _scope = nc.named_scope(f"P{2 + layer}_rec{layer}")
                _scope.__enter__()
                rng = ring[layer]
                rcols = RING0 if layer == 0 else RING1
                whh_sb = whh_tile(f"whh{layer}_sb")
                nc.sync.dma_start(
                    out=whh_sb[:],
                    in_=whh_d[layer][:].rearrange("(kb p) n -> p kb n", p=128),
                )
                for t in range(NT):
                    xw_t = stream.tile([32, 1024], F32, name="xw_t", tag="xwin", bufs=2)
                    nc.sync.dma_start(
                        out=xw_t[:], in_=xw_d[layer][t * 32 : (t + 1) * 32, :]
                    )
                    pcs = [psum_c(0, 32), psum_c(1, 32)]
                    rc_prev = ((t - 1) * 32) % rcols
                    for i in range(8):
                        stat = (
                            hT0[:, layer, i, :]
                            if t == 0
                            else rng[:, i, rc_prev : rc_prev + 32]
                        )
                        for jc in (0, 1):
                            nc.tensor.matmul(
                                pcs[jc][:],
                                stat,
                                whh_sb[:, i, jc * 512 : (jc + 1) * 512],
                                start=(i == 0),
                                stop=(i == 7),
                            )
                    h_sb = stream.tile([32, 1024], F32, name="h_sb", tag="h", bufs=2)
                    for jc in (0, 1):
                        nc.vector.tensor_add(
                            out=h_sb[:, jc * 512 : (jc + 1) * 512],
                            in0=pcs[jc][:],
                            in1=xw_t[:, jc * 512 : (jc + 1) * 512],
                        )
                        nc.scalar.activation(
                            h_sb[:, jc * 512 : (jc + 1) * 512],
                            h_sb[:, jc * 512 : (jc + 1) * 512],
                            Tanh,
                        )
                    rc = (t * 32) % rcols
                    for g in (0, 1):
                        pt = psum_t(g)
                        for i in range(4):
                            blk = g * 4 + i
                            nc.tensor.transpose(
                                out=pt[:, i * 32 : (i + 1) * 32],
                                in_=h_sb[:, blk * 128 : (blk + 1) * 128],
                                identity=ident[:32, :32],
                            )
                        nc.vector.tensor_copy(
                            out=rng[:, g * 4 : (g + 1) * 4, rc : rc + 32],
                            in_=pt[:, : 4 * 32].rearrange("p (a b) -> p a b", a=4),
                        )
                    if layer == 0 and t % 4 == 3:
                        k = t // 4
                        base = (k * 128) % RING0
                        proj_tile(
                            k,
                            lambda i, base=base: rng[:, i, base : base + 128],
                            wih1_sb,
                            biasl[1][:],
                            xw_d[1],
                        )
                    if layer == 1 and t % 8 == 7:
                        dbase = ((t - 7) * 32) % RING1
                        gbase = (t - 7) * 32
                        for blk in range(8):
                            nc.sync.dma_start(
                                out=y1t_d[blk, :, gbase : gbase + 256],
                                in_=rng[:, blk, dbase : dbase + 256],
                            )
                # final hidden capture (cast F32R ring -> F32 out)
                fc = ((NT - 1) * 32) % rcols
                hcap = stream.tile([128, 8, 32], F32, name="hcap", tag="hcap", bufs=1)
                nc.vector.tensor_copy(
                    out=hcap[:], in_=rng[:, :, fc : fc + 32]
                )
                nc.sync.dma_start(out=rnnh_d[layer], in_=hcap[:])
                _scope.__exit__(None, None, None)

            # =============== P4: logits GEMM (vocab shard)
            _scope = nc.named_scope("P4_logits")
            _scope.__enter__()
            for vc in range(NVC):
                wout_sb = wbig_tile("wout_sb")
                nc.sync.dma_start(
                    out=wout_sb[:],
                    in_=wout_d[:, vc * 1024 : (vc + 1) * 1024].rearrange(
                        "(kb p) n -> p kb n", p=128
                    ),
                )
                bout_sb = stream.tile(
                    [128, 1024], F32, name="bout_sb", tag="bout", bufs=1
                )
                nc.sync.dma_start(
                    out=bout_sb[:], in_=bout_d[:, vc * 1024 : (vc + 1) * 1024]
                )
                for btg in range(NBTG):
                    gw = min(512, BTn - btg * 512)
                    y1tile = stream.tile(
                        [128, 8, 512], F32R, name="y1tile", tag="t8x", bufs=2
                    )
                    nc.sync.dma_start(
                        out=y1tile[:, :, :gw],
                        in_=y1t_d[:, :, btg * 512 : btg * 512 + gw].rearrange(
                            "blk p c -> p blk c"
                        ),
                    )
                    for sub in range(gw // 128):
                        pcs = [psum_c(0), psum_c(1)]
                        for i in range(8):
                            stat = y1tile[:, i, sub * 128 : (sub + 1) * 128]
                            for jc in (0, 1):
                                nc.tensor.matmul(
                                    pcs[jc][:],
                                    stat,
                                    wout_sb[:, i, jc * 512 : (jc + 1) * 512],
                                    start=(i == 0),
                                    stop=(i == 7),
                                )
                        out_sb = stream.tile(
                            [128, 1024], F32, name="lout", tag="buf4k", bufs=3
                        )
                        for jc in (0, 1):
                            nc.vector.tensor_add(
                                out=out_sb[:, jc * 512 : (jc + 1) * 512],
                                in0=pcs[jc][:],
                                in1=bout_sb[:, jc * 512 : (jc + 1) * 512],
                            )
                        # rows are time-major t*32+b; store to b-major rows
                        lt = btg * 4 + sub  # 128-row tile index
                        for tt in range(4):
                            trow = lt * 4 + tt
                            nc.sync.dma_start(
                                out=logits_bt[:, trow, vc * 1024 : (vc + 1) * 1024],
                                in_=out_sb[tt * 32 : (tt + 1) * 32, :],
                            )
            _scope.__exit__(None, None, None)
    return nc


# ---------------------------------------------------------------------------
# Host entry point
# ---------------------------------------------------------------------------


def _prep_inputs(input_x, hidden, emb, W_ih0, W_hh0, b_ih0, b_hh0,
                 W_ih1, W_hh1, b_ih1, b_hh1, W_out, b_out, nsteps=T):
    f32 = np.float32
    NT = nsteps
    idx_t = np.ascontiguousarray(
        np.asarray(input_x).astype(np.int32)[:, :NT].T.reshape(-1, 1)
    )
    # hT_init[p, l, blk, b] must equal hidden[l, b, blk*128 + p]
    hT_init = np.ascontiguousarray(
        np.asarray(hidden, f32).transpose(0, 2, 1).reshape(2, 8, 128, 32)
        .transpose(2, 0, 1, 3)
    )
    common = {
        "idx_t": idx_t,
        "emb": np.ascontiguousarray(np.asarray(emb, f32)),
        "hT_init": np.ascontiguousarray(hT_init),
        "wihT0": np.ascontiguousarray(np.asarray(W_ih0, f32).T),
        "whhT0": np.ascontiguousarray(np.asarray(W_hh0, f32).T),
        "wihT1": np.ascontiguousarray(np.asarray(W_ih1, f32).T),
        "whhT1": np.ascontiguousarray(np.asarray(W_hh1, f32).T),
        "bias0": np.ascontiguousarray(
            np.tile((np.asarray(b_ih0, f32) + np.asarray(b_hh0, f32))[None, :],
                    (128, 1))
        ),
        "bias1": np.ascontiguousarray(
            np.tile((np.asarray(b_ih1, f32) + np.asarray(b_hh1, f32))[None, :],
                    (128, 1))
        ),
    }
    woutT_pad = np.zeros((H, VPAD), f32)
    woutT_pad[:, :V] = np.asarray(W_out, f32).T
    bout_pad = np.zeros(VPAD, f32)
    bout_pad[:V] = np.asarray(b_out, f32)
    in_maps = []
    for c in range(NCORES):
        m = dict(common)
        m["woutT"] = np.ascontiguousarray(woutT_pad[:, c * VSH : (c + 1) * VSH])
        m["bout"] = np.ascontiguousarray(
            np.tile(bout_pad[None, c * VSH : (c + 1) * VSH], (128, 1))
        )
        in_maps.append(m)
    return in_maps


_NC_CACHE = {}


def _enable_ldw_opt():
    """Compile with walrus LDWEIGHTS double-buffer optimization enabled.

    bass_utils hardcodes --enable-ldw-opt=false; ~40% of PE time in this
    kernel is weight loads, which the optimization overlaps with matmuls.
    Verified numerically identical on this workload.
    """
    import concourse.bass_utils as bu

    if getattr(bu.run_command, "_ldw_patched", False):
        return
    orig = bu.run_command

    def patched(argv, **kwargs):
        argv = [
            a.replace("--enable-ldw-opt=false", "--enable-ldw-opt=true")
            if isinstance(a, str) else a
            for a in argv
        ]
        return orig(argv, **kwargs)

    patched._ldw_patched = True
    bu.run_command = patched


def kernel(input_x, hidden, emb, W_ih0, W_hh0, b_ih0, b_hh0,
           W_ih1, W_hh1, b_ih1, b_hh1, W_out, b_out):
    from concourse.bass_utils import run_bass_kernel_spmd

    _enable_ldw_opt()

    in_maps = _prep_inputs(
        input_x, hidden, emb, W_ih0, W_hh0, b_ih0, b_hh0,
        W_ih1, W_hh1, b_ih1, b_hh1, W_out, b_out,
    )
    if "nc" not in _NC_CACHE:
        nc = _build_nc()
        _split_excess_waits(nc)
        _NC_CACHE["nc"] = nc
    nc = _NC_CACHE["nc"]
    res = run_bass_kernel_spmd(nc, in_maps, list(range(NCORES)), trace=False)
    results = res.results

    output_prob = np.concatenate(
        [results[c]["logits"] for c in range(NCORES)], axis=1
    )[:, :V]
    rnnh = results[0]["rnnh"]  # [2, 128, 8, 32] = [l, p, blk, b]
    rnn_hidden = np.ascontiguousarray(
        rnnh.transpose(0, 3, 2, 1).reshape(2, B, H)
    )
    last_output = np.ascontiguousarray(
        output_prob.reshape(B, T, V)[:, -1, :]
    )
    return (output_prob, rnn_hidden, last_output)


# revision 13
# speedup vs baseline: 1.0461x; 1.0356x over previous
"""Trainium2 Bass kernel for the CS224N 2-layer RNN language model.

Model: x = emb[idx]; two tanh-RNN layers; logits = y1 @ W_out.T + b_out.
Returns (output_prob [B*T,V], rnn_hidden [2,B,H], last_output [B,V]).

Strategy (8 NeuronCores, SPMD):
  - The RNN (embedding gather, both input projections, both recurrences)
    is replicated on every core: the sequential recurrence is bound by
    streaming W_hh through the PE array each step, which cannot be
    usefully split across cores without per-step collectives (measured
    ~21us latency per collective here - prohibitive).
  - The dominant [4096,1024]@[1024,32000] logits GEMM is tensor-parallel
    sharded over the vocab dim: W_out is padded to 32768 cols and each
    core computes a 4096-col shard. Host concatenates shards.
  - All GEMMs run as float32r (TF32-like, 1 cycle/row at N>=512;
    measured mean rel err ~1e-3, vs 4 cycles/row for full fp32).
  - Internal time-major layout (row = t*B + b) so per-step slices and
    projection tiles are contiguous; transposed activations (hT blocks)
    are produced on the PE (transpose mode) each step and double as both
    the next step's stationary operand and the next layer's input.

Self-contained: hardcodes all shapes; no sibling imports.
"""

import numpy as np

V, E, H, B, T = 32000, 1024, 1024, 32, 128
VPAD = 32768
NCORES = 8
VSH = VPAD // NCORES  # 4096 per-core vocab shard

# ---------------------------------------------------------------------------
# BIR post-pass: the walrus build in this image encodes at most ONE semaphore
# wait per instruction; Tile attaches many to its kernel-tail drain. Move the
# excess onto NoOps inserted immediately before, same engine (same semantics).
# ---------------------------------------------------------------------------
_wsplit_counter = [0]


def _split_excess_waits(nc, limit=1):
    import bass_rust
    import concourse.mybir as mybir

    for func in nc.m.functions:
        for bb in func.blocks:
            il = bb.instructions
            i = 0
            while i < len(il):
                inst = il[i]
                si = getattr(inst, "sync_info", None)
                if si is None:
                    i += 1
                    continue
                w = list(si.on_wait)
                if len(w) > limit:
                    keep = w[-limit:]
                    excess = w[:-limit]
                    si.on_wait = keep
                    ncarr = 0
                    for j in range(0, len(excess), limit):
                        _wsplit_counter[0] += 1
                        nop = mybir.InstNoOp(
                            name=f"WSPLIT-{_wsplit_counter[0]}", ins=[], outs=[]
                        )
                        nop.engine = inst.engine
                        nop.sync_info = bass_rust.SyncInfo(
                            on_wait=excess[j : j + limit], on_update=[]
                        )
                        il.insert(i + ncarr, nop)
                        ncarr += 1
                    i += ncarr
                i += 1


# ---------------------------------------------------------------------------
# Device program
# ---------------------------------------------------------------------------


def _build_nc(nsteps=T, debug_outputs=False):
    import concourse.bass as bass
    import concourse.mybir as mybir
    import concourse.tile as tile
    from concourse.masks import make_identity

    F32 = mybir.dt.float32
    F32R = mybir.dt.float32r
    BF16 = mybir.dt.bfloat16
    I32 = mybir.dt.int32
    Tanh = mybir.ActivationFunctionType.Tanh

    NT = nsteps
    BTn = NT * B  # rows, time-major
    NPROJ = BTn // 128  # projection tiles (128 rows each)
    RING0 = 512  # cols in y0T ring (16 steps of history)
    RING1 = 256
    NBTG = (BTn + 511) // 512  # logits bt groups of <=512 rows
    NVC = 4  # logits vocab chunks of 1024

    nc = bass.Bass()

    okind = "ExternalOutput" if debug_outputs else "Internal"

    idx_d = nc.dram_tensor("idx_t", [BTn, 1], I32, kind="ExternalInput")
    emb_d = nc.dram_tensor("emb", [V, E], F32, kind="ExternalInput")
    hT_init_d = nc.dram_tensor("hT_init", [128, 2, 8, 32], F32R, kind="ExternalInput")
    wih_d = [
        nc.dram_tensor(f"wihT{l}", [E, H], F32R, kind="ExternalInput") for l in (0, 1)
    ]
    whh_d = [
        nc.dram_tensor(f"whhT{l}", [H, H], F32R, kind="ExternalInput") for l in (0, 1)
    ]
    bias_d = [
        nc.dram_tensor(f"bias{l}", [128, H], F32, kind="ExternalInput") for l in (0, 1)
    ]
    wout_d = nc.dram_tensor("woutT", [H, VSH], BF16, kind="ExternalInput")
    bout_d = nc.dram_tensor("bout", [128, VSH], F32, kind="ExternalInput")

    logits_d = nc.dram_tensor("logits", [BTn, VSH], F32, kind="ExternalOutput")
    rnnh_d = nc.dram_tensor("rnnh", [2, 128, 8, 32], F32, kind="ExternalOutput")

    xw_d = [nc.dram_tensor(f"xw{l}", [BTn, H], F32, kind=okind) for l in (0, 1)]
    y1t_d = nc.dram_tensor("y1t", [8, 128, BTn], BF16, kind=okind)

    # output rows are b-major (row b*NT + t) to match the reference layout
    logits_bt = logits_d[:].rearrange("(b t) v -> b t v", t=NT)

    with tile.TileContext(nc) as tc:
        with (
            tc.tile_pool(name="const", bufs=1) as constp,
            tc.tile_pool(name="wpool", bufs=1) as wpool,
            tc.tile_pool(name="stream", bufs=1) as stream,
            tc.tile_pool(name="ps", bufs=1, space="PSUM") as ps,
        ):
            ident = constp.tile([128, 128], F32)
            make_identity(nc, ident[:])
            ring = [
                constp.tile([128, 8, RING0], F32R, name="ring0"),
                constp.tile([128, 8, RING1], F32R, name="ring1"),
            ]
            y1acc = constp.tile([128, 8, 512], BF16, name="y1acc")
            hT0 = constp.tile([128, 2, 8, 32], F32R, name="hT0")
            nc.sync.dma_start(out=hT0[:], in_=hT_init_d[:])
            biasl = []
            for l in (0, 1):
                bl = constp.tile([128, H], F32, name=f"biasl{l}")
                nc.sync.dma_start(out=bl[:], in_=bias_d[l][:])
                biasl.append(bl)

            def wbig_tile(name):
                return wpool.tile([128, 8, 1024], F32R, name=name, tag="wbig", bufs=2)

            def whh_tile(name):
                return wpool.tile([128, 8, 1024], F32R, name=name, tag="whh", bufs=1)

            def psum_c(jc, mrows=128):
                return ps.tile(
                    [mrows, 512], mybir.dt.float32, space="PSUM",
                    tag=f"pc{jc}", name=f"pc{jc}", bufs=2,
                )

            def psum_t(g):
                return ps.tile(
                    [128, 512], mybir.dt.float32, space="PSUM",
                    tag=f"pt{g}", name=f"pt{g}", bufs=2,
                )

            def proj_tile(k, stat, w_sb, bias_sb, out_dram):
                """out rows k*128..k*128+128 = stat.T @ w  (+bias)."""
                pcs = [psum_c(0), psum_c(1)]
                for i in range(8):
                    for jc in (0, 1):
                        nc.tensor.matmul(
                            pcs[jc][:],
                            stat(i),
                            w_sb[:, i, jc * 512 : (jc + 1) * 512],
                            start=(i == 0),
                            stop=(i == 7),
                        )
                out_sb = stream.tile(
                    [128, 1024], F32, name="projout", tag="buf4k", bufs=3
                )
                for jc in (0, 1):
                    nc.vector.tensor_add(
                        out=out_sb[:, jc * 512 : (jc + 1) * 512],
                        in0=pcs[jc][:],
                        in1=bias_sb[:, jc * 512 : (jc + 1) * 512],
                    )
                nc.sync.dma_start(
                    out=out_dram[k * 128 : (k + 1) * 128, :], in_=out_sb[:]
                )

            # =============== P1: embedding gather + L0 input projection
            _scope = nc.named_scope("P1_proj0")
            _scope.__enter__()
            wih0_sb = wbig_tile("wih0_sb")
            nc.sync.dma_start(
                out=wih0_sb[:], in_=wih_d[0][:].rearrange("(kb p) n -> p kb n", p=128)
            )
            for k in range(NPROJ):
                idx_sb = stream.tile([128, 1], I32, name="idx_sb", tag="idx", bufs=2)
                nc.sync.dma_start(out=idx_sb[:], in_=idx_d[k * 128 : (k + 1) * 128, :])
                xg = stream.tile([128, 1024], F32, name="xg", tag="buf4k", bufs=3)
                nc.gpsimd.indirect_dma_start(
                    out=xg[:],
                    out_offset=None,
                    in_=emb_d[:],
                    in_offset=bass.IndirectOffsetOnAxis(ap=idx_sb[:, :1], axis=0),
                )
                xT = stream.tile([128, 8, 512], F32R, name="xT", tag="t8x", bufs=2)
                for g in (0, 1):
                    pt = psum_t(g)
                    for i in range(4):
                        blk = g * 4 + i
                        nc.tensor.transpose(
                            out=pt[:, i * 128 : (i + 1) * 128],
                            in_=xg[:, blk * 128 : (blk + 1) * 128],
                            identity=ident[:],
                        )
                    nc.vector.tensor_copy(
                        out=xT[:, g * 4 : (g + 1) * 4, :128],
                        in_=pt[:].rearrange("p (a b) -> p a b", a=4),
                    )
                proj_tile(
                    k,
                    lambda i, xT=xT: xT[:, i, :128],
                    wih0_sb,
                    biasl[0][:],
                    xw_d[0],
                )

            _scope.__exit__(None, None, None)

            # =============== P2/P3: recurrences (+ interleaved L1 projection)
            wih1_sb = wbig_tile("wih1_sb")
            nc.sync.dma_start(
                out=wih1_sb[:], in_=wih_d[1][:].rearrange("(kb p) n -> p kb n", p=128)
            )

            for layer in (0, 1):
                _scope = nc.named_scope(f"P{2 + layer}_rec{layer}")
                _scope.__enter__()
                rng = ring[layer]
                rcols = RING0 if layer == 0 else RING1
                whh_sb = whh_tile(f"whh{layer}_sb")
                nc.sync.dma_start(
                    out=whh_sb[:],
                    in_=whh_d[layer][:].rearrange("(kb p) n -> p kb n", p=128),
                )
                for t in range(NT):
                    xw_t = stream.tile([32, 1024], F32, name="xw_t", tag="xwin", bufs=2)
                    nc.sync.dma_start(
                        out=xw_t[:], in_=xw_d[layer][t * 32 : (t + 1) * 32, :]
                    )
                    pcs = [psum_c(0, 32), psum_c(1, 32)]
                    rc_prev = ((t - 1) * 32) % rcols
                    for i in range(8):
                        stat = (
                            hT0[:, layer, i, :]
                            if t == 0
                            else rng[:, i, rc_prev : rc_prev + 32]
                        )
                        for jc in (0, 1):
                            nc.tensor.matmul(
                                pcs[jc][:],
                                stat,
                                whh_sb[:, i, jc * 512 : (jc + 1) * 512],
                                start=(i == 0),
                                stop=(i == 7),
                            )
                    h_sb = stream.tile([32, 1024], F32, name="h_sb", tag="h", bufs=2)
                    for jc in (0, 1):
                        nc.vector.tensor_add(
                            out=h_sb[:, jc * 512 : (jc + 1) * 512],
                            in0=pcs[jc][:],
                            in1=xw_t[:, jc * 512 : (jc + 1) * 512],
                        )
                        nc.scalar.activation(
                            h_sb[:, jc * 512 : (jc + 1) * 512],
                            h_sb[:, jc * 512 : (jc + 1) * 512],
                            Tanh,
                        )
                    rc = (t * 32) % rcols
                    for g in (0, 1):
                        pt = psum_t(g)
                        for i in range(4):
                            blk = g * 4 + i
                            nc.tensor.transpose(
                                out=pt[:, i * 32 : (i + 1) * 32],
                                in_=h_sb[:, blk * 128 : (blk + 1) * 128],
                                identity=ident[:32, :32],
                            )
                        nc.vector.tensor_copy(
                            out=rng[:, g * 4 : (g + 1) * 4, rc : rc + 32],
                            in_=pt[:, : 4 * 32].rearrange("p (a b) -> p a b", a=4),
                        )
                        if layer == 1:
                            ya = (t * 32) % 512
                            nc.vector.tensor_copy(
                                out=y1acc[:, g * 4 : (g + 1) * 4, ya : ya + 32],
                                in_=pt[:, : 4 * 32].rearrange(
                                    "p (a b) -> p a b", a=4
                                ),
                            )
                    if layer == 0 and t % 4 == 3:
                        k = t // 4
                        base = (k * 128) % RING0
                        proj_tile(
                            k,
                            lambda i, base=base: rng[:, i, base : base + 128],
                            wih1_sb,
                            biasl[1][:],
                            xw_d[1],
                        )
                    if layer == 1 and t % 8 == 7:
                        dbase = ((t - 7) * 32) % 512
                        gbase = (t - 7) * 32
                        for blk in range(8):
                            nc.sync.dma_start(
                                out=y1t_d[blk, :, gbase : gbase + 256],
                                in_=y1acc[:, blk, dbase : dbase + 256],
                            )
                # final hidden capture (cast F32R ring -> F32 out)
                fc = ((NT - 1) * 32) % rcols
                hcap = stream.tile([128, 8, 32], F32, name="hcap", tag="hcap", bufs=1)
                nc.vector.tensor_copy(
                    out=hcap[:], in_=rng[:, :, fc : fc + 32]
                )
                nc.sync.dma_start(out=rnnh_d[layer], in_=hcap[:])
                _scope.__exit__(None, None, None)

            # =============== P4: logits GEMM (vocab shard)
            _scope = nc.named_scope("P4_logits")
            _scope.__enter__()
            for vc in range(NVC):
                wout_sb = wpool.tile(
                    [128, 8, 1024], BF16, name="wout_sb", tag="wbig", bufs=2
                )
                nc.sync.dma_start(
                    out=wout_sb[:],
                    in_=wout_d[:, vc * 1024 : (vc + 1) * 1024].rearrange(
                        "(kb p) n -> p kb n", p=128
                    ),
                )
                bout_sb = stream.tile(
                    [128, 1024], F32, name="bout_sb", tag="bout", bufs=1
                )
                nc.sync.dma_start(
                    out=bout_sb[:], in_=bout_d[:, vc * 1024 : (vc + 1) * 1024]
                )
                for btg in range(NBTG):
                    gw = min(512, BTn - btg * 512)
                    y1tile = stream.tile(
                        [128, 8, 512], BF16, name="y1tile", tag="t8x", bufs=2
                    )
                    nc.sync.dma_start(
                        out=y1tile[:, :, :gw],
                        in_=y1t_d[:, :, btg * 512 : btg * 512 + gw].rearrange(
                            "blk p c -> p blk c"
                        ),
                    )
                    for sub in range(gw // 128):
                        pcs = [psum_c(0), psum_c(1)]
                        for i in range(8):
                            stat = y1tile[:, i, sub * 128 : (sub + 1) * 128]
                            for jc in (0, 1):
                                nc.tensor.matmul(
                                    pcs[jc][:],
                                    stat,
                                    wout_sb[:, i, jc * 512 : (jc + 1) * 512],
                                    start=(i == 0),
                                    stop=(i == 7),
                                )
                        out_sb = stream.tile(
                            [128, 1024], F32, name="lout", tag="buf4k", bufs=3
                        )
                        for jc in (0, 1):
                            nc.vector.tensor_add(
                                out=out_sb[:, jc * 512 : (jc + 1) * 512],
                                in0=pcs[jc][:],
                                in1=bout_sb[:, jc * 512 : (jc + 1) * 512],
                            )
                        # rows are time-major t*32+b; store to b-major rows
                        lt = btg * 4 + sub  # 128-row tile index
                        for tt in range(4):
                            trow = lt * 4 + tt
                            nc.sync.dma_start(
                                out=logits_bt[:, trow, vc * 1024 : (vc + 1) * 1024],
                                in_=out_sb[tt * 32 : (tt + 1) * 32, :],
                            )
            _scope.__exit__(None, None, None)
    return nc


# ---------------------------------------------------------------------------
# Host entry point
# ---------------------------------------------------------------------------


def _prep_inputs(input_x, hidden, emb, W_ih0, W_hh0, b_ih0, b_hh0,
                 W_ih1, W_hh1, b_ih1, b_hh1, W_out, b_out, nsteps=T):
    f32 = np.float32
    NT = nsteps
    idx_t = np.ascontiguousarray(
        np.asarray(input_x).astype(np.int32)[:, :NT].T.reshape(-1, 1)
    )
    # hT_init[p, l, blk, b] must equal hidden[l, b, blk*128 + p]
    hT_init = np.ascontiguousarray(
        np.asarray(hidden, f32).transpose(0, 2, 1).reshape(2, 8, 128, 32)
        .transpose(2, 0, 1, 3)
    )
    common = {
        "idx_t": idx_t,
        "emb": np.ascontiguousarray(np.asarray(emb, f32)),
        "hT_init": np.ascontiguousarray(hT_init),
        "wihT0": np.ascontiguousarray(np.asarray(W_ih0, f32).T),
        "whhT0": np.ascontiguousarray(np.asarray(W_hh0, f32).T),
        "wihT1": np.ascontiguousarray(np.asarray(W_ih1, f32).T),
        "whhT1": np.ascontiguousarray(np.asarray(W_hh1, f32).T),
        "bias0": np.ascontiguousarray(
            np.tile((np.asarray(b_ih0, f32) + np.asarray(b_hh0, f32))[None, :],
                    (128, 1))
        ),
        "bias1": np.ascontiguousarray(
            np.tile((np.asarray(b_ih1, f32) + np.asarray(b_hh1, f32))[None, :],
                    (128, 1))
        ),
    }
    woutT_pad = np.zeros((H, VPAD), f32)
    woutT_pad[:, :V] = np.asarray(W_out, f32).T
    bout_pad = np.zeros(VPAD, f32)
    bout_pad[:V] = np.asarray(b_out, f32)
    in_maps = []
    for c in range(NCORES):
        m = dict(common)
        import ml_dtypes

        m["woutT"] = np.ascontiguousarray(
            woutT_pad[:, c * VSH : (c + 1) * VSH]
        ).astype(ml_dtypes.bfloat16)
        m["bout"] = np.ascontiguousarray(
            np.tile(bout_pad[None, c * VSH : (c + 1) * VSH], (128, 1))
        )
        in_maps.append(m)
    return in_maps


_NC_CACHE = {}


def _enable_ldw_opt():
    """Compile with walrus LDWEIGHTS double-buffer optimization enabled.

    bass_utils hardcodes --enable-ldw-opt=false; ~40% of PE time in this
    kernel is weight loads, which the optimization overlaps with matmuls.
    Verified numerically identical on this workload.
    """
    import concourse.bass_utils as bu

    return  # ldw-opt=true crashes walrus codegen on real (bf16) InstLdweights
    if getattr(bu.run_command, "_ldw_patched", False):
        return
    orig = bu.run_command

    def patched(argv, **kwargs):
        argv = [
            a.replace("--enable-ldw-opt=false", "--enable-ldw-opt=true")
            if isinstance(a, str) else a
            for a in argv
        ]
        return orig(argv, **kwargs)

    patched._ldw_patched = True
    bu.run_command = patched


def kernel(input_x, hidden, emb, W_ih0, W_hh0, b_ih0, b_hh0,
           W_ih1, W_hh1, b_ih1, b_hh1, W_out, b_out):
    from concourse.bass_utils import run_bass_kernel_spmd

    _enable_ldw_opt()

    in_maps = _prep_inputs(
        input_x, hidden, emb, W_ih0, W_hh0, b_ih0, b_hh0,
        W_ih1, W_hh1, b_ih1, b_hh1, W_out, b_out,
    )
    if "nc" not in _NC_CACHE:
        nc = _build_nc()
        _split_excess_waits(nc)
        _NC_CACHE["nc"] = nc
    nc = _NC_CACHE["nc"]
    res = run_bass_kernel_spmd(nc, in_maps, list(range(NCORES)), trace=False)
    results = res.results

    output_prob = np.concatenate(
        [results[c]["logits"] for c in range(NCORES)], axis=1
    )[:, :V]
    rnnh = results[0]["rnnh"]  # [2, 128, 8, 32] = [l, p, blk, b]
    rnn_hidden = np.ascontiguousarray(
        rnnh.transpose(0, 3, 2, 1).reshape(2, B, H)
    )
    last_output = np.ascontiguousarray(
        output_prob.reshape(B, T, V)[:, -1, :]
    )
    return (output_prob, rnn_hidden, last_output)
